# revision 44
# baseline (speedup 1.0000x reference)
"""Distributed attention kernel for one TRN2 chip (8 NeuronCores), v4.

Sharding: 16 heads / 8 cores = 2 heads per core (head-group parallel).
No collectives: each core computes a full [BT, C] PARTIAL of the output
projection from its 2 heads (contracting its 128-row slice of w_proj)
and the host sums the 8 bf16 partials (+ b_proj) during unshard.

Trace-driven findings baked in (vs the v2 baseline):
  - the exp stream on ACT is the pacer: back-to-back exp of a
    [128,2,512] S tile sustains ~1.0us; the kernel is PE-bound at ~94%
    occupancy around it, so injected QKV/proj granule placement matters
    more than anything else.
  - x ships as contiguous half-regions (4-8KB/partition lines, one
    trigger each, qk-critical ones split in c-halves) on the sync HWDGE
    ring, weights in deadline order ahead of the x region they gate.
    48 small strided chunks + 623ns/trigger serialization previously
    delayed the first x bytes to 12.8us.
  - 12 small N=128 junk matmuls warm the HAM clock during the x wait
    without parking cold N=512 work at the head of the PE FIFO (16 such
    burned 10us at 1.2GHz and delayed everything).
  - injected tasks: one ~0.5-0.9us granule per step max; unit-boundary
    steps (s%16 in {0,1}) always carry one so the PE has queued work
    while the drain's oc copy releases the op PSUM banks.
  - the last unit's drain+proj pipeline is split per token chunk with
    casts split across DVE and ACT (idle after the last exp), the
    reciprocal runs 128-partitions wide, and junk matmuls keep the HAM
    clock warm through the tail.

Per core:
  - QKV: Q^T,K^T head-dim-major; V token-major with 64 duplicated ones
    columns per head, so the PV matmul emits the softmax denominator
    broadcast across PSUM rows 64-127 and normalization is 2 DVE ops
  - attention as one flat software pipeline over 128 (unit, kc) steps,
    unit = (batch, 512-query chunk): dual row-tiled score matmuls
    (head0 on PE rows 0-63, head1 on rows 64-127 run concurrently,
    leading each step's emission under tc.high_priority), exp on ACT,
    PV accumulation
  - proj partial: ot^T token chunks (stationary) x own w_proj rows ->
    [128 tok, 1024] fp32 -> bf16 -> DMA out on gpsimd/sync

PSUM budget (8 banks): S-pair ring 2x[128,2,512] (4) + PV/denominator
accumulator [128,2,512] (2) + qkv/proj scratch ring 2x[128,512] (2).
"""

import numpy as np

_CACHE = {}

P = 128
B, T, C = 2, 2048, 1024
BT = B * T
NCORE = 8
HD = 64  # head dim
CSL = 128  # per-core c-slice = 2 heads * 64
TQ = 512  # query chunk
NQC = T // TQ  # 4
KC = 128  # key chunk (partition dim)
NKC = T // KC  # 16
NCC = C // P  # 8 contraction chunks
NTC = BT // P  # 32 token chunks of 128
TB = T // P  # 16 token chunks per batch
NHR = 8  # x half-regions of 512 tokens, hr == t8 chunk index
VW = 128  # per-head V stationary: 64 v cols + 64 ones cols (denominator
# lands broadcast across PSUM rows 64-127 straight out of the PV matmul)


def _build():
    import concourse.bass as bass
    import concourse.tile as tile
    from concourse import bacc, mybir

    F32 = mybir.dt.float32
    BF16 = mybir.dt.bfloat16
    Exp = mybir.ActivationFunctionType.Exp

    nc = bacc.Bacc("TRN2", target_bir_lowering=False, debug=False, num_devices=NCORE)

    # x: [hr, p, c*512] - each hr slice is one contiguous 1MB DMA with
    # 8KB per-partition lines (descriptor-efficient, ~line rate)
    x_ext = nc.declare_dram_parameter("x", [NHR, P, NCC * TQ], BF16, isOutput=False)
    # all three projection weights in one tensor, sliced per-DMA
    wqkv_ext = nc.declare_dram_parameter("wqkv", [P, 3, NCC, CSL], BF16, isOutput=False)
    wp_ext = nc.declare_dram_parameter("wp", [CSL, C], BF16, isOutput=False)
    bqk_ext = nc.declare_dram_parameter("bqk", [CSL, 2], F32, isOutput=False)
    bv_ext = nc.declare_dram_parameter("bv", [1, CSL], F32, isOutput=False)
    out_ext = nc.declare_dram_parameter("out", [BT, C], BF16, isOutput=True)

    with tile.TileContext(nc) as tc:
        with (
            nc.allow_low_precision("bf16 attention compute by design"),
            tc.tile_pool(name="pers", bufs=1) as pers,
            tc.tile_pool(name="stage", bufs=3) as stage,
            tc.tile_pool(name="ptp", bufs=4) as ptp,
            tc.tile_pool(name="psm", bufs=1, space="PSUM") as psm,
        ):
            # ---- persistent SBUF tiles ----
            xt_sb = pers.tile([P, NHR, NCC * TQ], BF16, name="xt_sb")
            qt_sb = pers.tile([P, BT], BF16, name="qt_sb")  # Q^T (rows: 2*64 head dims)
            kt_sb = pers.tile([P, BT], BF16, name="kt_sb")
            v_sb = pers.tile([P, NTC, 2, VW], BF16, name="v_sb")  # [tok, chunk, head, vcol]
            wqkv_sb = pers.tile([P, 3, NCC, CSL], BF16, name="wqkv_sb")
            ot_sb = pers.tile([P, BT], BF16, name="ot_sb")  # attention out^T (c-slice rows)
            wp_sb = pers.tile([P, C], BF16, name="wp_sb")  # own 128 rows of w_proj
            bqk_sb = pers.tile([CSL, 2], F32, name="bqk_sb")
            bv_row = pers.tile([1, CSL], F32, name="bv_row")
            bv_bc = pers.tile([P, CSL], F32, name="bv_bc")
            ones1 = pers.tile([1, TQ], F32, name="ones1")
            wsrc = pers.tile([P, TQ], BF16, name="wsrc")

            nc.gpsimd.memset(ones1[:], 1.0)
            nc.gpsimd.memset(wsrc[:], 1.0)
            # denominator ones columns (gpsimd queue is otherwise idle at
            # startup; all input DMAs ride the HWDGE rings)
            nc.gpsimd.memset(v_sb[:, :, :, HD:VW], 1.0)

            WQ, WK, WV = 0, 1, 2  # first index of wqkv_sb

            # input DMAs on the sync HWDGE ring in strict deadline order
            # (a second ring only dilutes the bandwidth of the critical
            # early bytes - the early-HBM phase sustains ~100-150GB/s
            # with all 8 cores starting up). The qk-critical x regions
            # ship in c-halves so each qk_part unblocks at 512KB
            # granularity. wp rides the scalar ring: needed last, and the
            # scalar engine is idle before the first exp.
            HTQ = NCC * TQ // 2
            nc.sync.dma_start(bqk_sb[:], bqk_ext[:])
            nc.sync.dma_start(bv_row[:], bv_ext[:])
            nc.sync.dma_start(wqkv_sb[:, WK:WK + 1], wqkv_ext[:, WK:WK + 1])
            nc.sync.dma_start(wqkv_sb[:, WQ:WQ + 1], wqkv_ext[:, WQ:WQ + 1])
            nc.sync.dma_start(xt_sb[:, 0, 0:HTQ], x_ext[0, :, 0:HTQ])
            nc.sync.dma_start(xt_sb[:, 0, HTQ:], x_ext[0, :, HTQ:])
            nc.sync.dma_start(wqkv_sb[:, WV:WV + 1], wqkv_ext[:, WV:WV + 1])
            for hr in range(1, 4):
                nc.sync.dma_start(xt_sb[:, hr, 0:HTQ], x_ext[hr, :, 0:HTQ])
                nc.sync.dma_start(xt_sb[:, hr, HTQ:], x_ext[hr, :, HTQ:])
            for hr in range(4, NHR):
                nc.sync.dma_start(xt_sb[:, hr, :], x_ext[hr])
            nc.scalar.dma_start(wp_sb[:], wp_ext[:])

            # junk matmuls bridging the PE from bring-up (~7.4us) to the
            # first x arrival (~15-17us): the HAM clock gate re-throttles
            # to 1.2GHz after a ~3.4us idle window, and a short bridge
            # puts the WHOLE prologue (kt0/qt0/S0) at half clock, pushing
            # exp(0) from ~20 to ~23.5us. N=512 junk issues at ~427ns
            # cold, so 18 of them cover ~7.7us.
            wt = psm.tile([P, 2, TQ], F32, tag="op", bufs=1, name="warm")
            for _ in range(18):
                nc.tensor.matmul(wt[:, 0, :], wsrc[:, 0:P], wsrc[:],
                                 start=True, stop=True)

            # preload the exp activation table (~2.7us) while DMAs run
            scr = stage.tile([1, 8], F32, tag="rc0", bufs=3, name="scr")
            nc.scalar.activation(scr[:], ones1[0:1, 0:8], Exp)

            def xt(c, t0, n):
                hr, off = divmod(t0, TQ)
                assert off + n <= TQ
                return xt_sb[:, hr, c * TQ + off:c * TQ + off + n]

            qk_ps = {}

            def qk_half(wi, dst, t8, half):
                # half a K/Q chunk (4 of 8 contraction MMs): injected
                # tasks must stay under ~1us of PE or they push the next
                # S pair out and the exp stream gaps 1-2 steps later
                key = (wi, t8)
                if half == 0:
                    qk_ps[key] = psm.tile([P, TQ], F32, tag="mm", bufs=2, name="ps_qk")
                for c in range(4 * half, 4 * half + 4):
                    nc.tensor.matmul(
                        qk_ps[key][:], wqkv_sb[:, wi, c, :], xt(c, t8 * TQ, TQ),
                        start=(c == 0), stop=(c == NCC - 1),
                    )
                if half == 1:
                    nc.vector.tensor_scalar_add(
                        dst[:, t8 * TQ:(t8 + 1) * TQ], qk_ps.pop(key)[:],
                        bqk_sb[:, wi:wi + 1])

            def v_chunk(i):
                ps = psm.tile([P, CSL], F32, tag="mm", bufs=2, name="ps_v")
                for c in range(NCC):
                    nc.tensor.matmul(
                        ps[:], xt(c, i * P, P), wqkv_sb[:, WV, c, :],
                        start=(c == 0), stop=(c == NCC - 1),
                    )
                nc.vector.tensor_add(v_sb[:, i, 0, 0:HD], ps[:, 0:HD], bv_bc[:, 0:HD])
                nc.vector.tensor_add(v_sb[:, i, 1, 0:HD], ps[:, HD:CSL], bv_bc[:, HD:CSL])

            def proj_chunk(t, tail=False, last_chunk=False):
                # partial projection for token chunk t: [128 tok, C] fp32
                ost = stage.tile([P, C], BF16, tag="ost", bufs=3, name="ost")
                for half in range(2):
                    pp = psm.tile([P, TQ], F32, tag="mm", bufs=2, name="pp")
                    nc.tensor.matmul(
                        pp[:], ot_sb[:, t * P:(t + 1) * P],
                        wp_sb[:, half * TQ:(half + 1) * TQ],
                        start=True, stop=True,
                    )
                    if tail:
                        # ACT (idle after the last exp) takes the tail
                        # casts: a DVE cast would wait on the proj matmul
                        # at the head of the DVE FIFO and block the next
                        # chunk's normalization muls queued behind it.
                        # Exception: the LAST chunk's first cast goes to
                        # the then-free DVE, shortening ACT's serial
                        # cast chain. The out DMA fires per half so the
                        # final transfer tail is a 256KB half.
                        if last_chunk and half == 0:
                            nc.vector.tensor_copy(
                                ost[:, half * TQ:(half + 1) * TQ], pp[:])
                        else:
                            nc.scalar.copy(ost[:, half * TQ:(half + 1) * TQ], pp[:])
                        eng = nc.gpsimd if (2 * t + half) % 2 == 0 else nc.sync
                        eng.dma_start(
                            out_ext[t * P:(t + 1) * P, half * TQ:(half + 1) * TQ],
                            ost[:, half * TQ:(half + 1) * TQ])
                    else:
                        nc.vector.tensor_copy(ost[:, half * TQ:(half + 1) * TQ], pp[:])
                if not tail:
                    eng = nc.gpsimd if t % 2 == 0 else nc.sync
                    eng.dma_start(out_ext[t * P:(t + 1) * P, :], ost[:])

            units = [(b, qc) for b in range(B) for qc in range(NQC)]
            NS = len(units) * NKC  # 128 pipeline steps

            sp_tiles = {}

            def do_S(s):
                # high priority: the exp stream (the pacer) is gated on
                # these, so the scheduler must sequence them ahead of any
                # ready PV/proj/qkv backlog on the PE queue
                u, k0 = divmod(s, NKC)
                b, qc = units[u]
                sp = psm.tile([P, 2, TQ], F32, tag="sp", bufs=2, name="sp")
                with tc.high_priority():
                    for h in range(2):
                        nc.tensor.matmul(
                            sp[:, h, :],
                            kt_sb[h * HD:(h + 1) * HD, b * T + k0 * KC: b * T + (k0 + 1) * KC],
                            qt_sb[h * HD:(h + 1) * HD, b * T + qc * TQ: b * T + (qc + 1) * TQ],
                            start=True, stop=True,
                        )
                sp_tiles[s] = sp

            def junk_fill():
                jt = psm.tile([P, TQ], F32, tag="mm", bufs=2, name="junk")
                nc.tensor.matmul(jt[:], wsrc[:, 0:P], xt_sb[:, 0, 0:TQ],
                                 start=True, stop=True)

            def drain(op_t, u, last=False):
                # op rows 64-127 hold 64 broadcast copies of the softmax
                # denominator (from the duplicated ones columns of the V
                # stationary). Copy the WHOLE accumulator to SBUF in one
                # op so the op PSUM bank pair is released after ~1us; the
                # reciprocal and normalization then run off the critical
                # path while the next unit's PVs accumulate.
                b, qc = units[u]
                base = b * T + qc * TQ
                if last:
                    # tail: nothing follows, so normalize straight out of
                    # PSUM, pipelined per token chunk into the final proj
                    # matmuls. The per-head denominator rows land on
                    # separate partition halves so the reciprocal runs
                    # 128-wide (full-width DVE ops: per-chunk splitting
                    # of the copies/recip was slower due to ~250ns DVE
                    # small-op overheads). Junk matmuls keep the PE (and
                    # its HAM clock) busy while the DVE computes scales.
                    rbf = stage.tile([P, TQ], F32, tag="oc", bufs=2, name="rbf")
                    # the h1 copy keeps its partition range (64-127) so it
                    # can ride the idle ACT engine concurrently with the
                    # DVE's partition-shifted h0 copy
                    nc.vector.tensor_copy(rbf[0:HD, :], op_t[HD:2 * HD, 0, :])
                    nc.scalar.copy(rbf[HD:P, :], op_t[HD:2 * HD, 1, :])
                    rb = stage.tile([P, TQ], F32, tag="rb", bufs=3, name="rb")
                    nc.vector.reciprocal_approx_fast(rb[:], rbf[:])
                    for _ in range(6):
                        junk_fill()
                    t0 = base // P
                    for j in range(TQ // P):
                        jP = j * P
                        for h in range(2):
                            nc.vector.tensor_mul(
                                ot_sb[h * HD:(h + 1) * HD, base + jP:base + jP + P],
                                op_t[0:HD, h, jP:jP + P],
                                rb[h * HD:(h + 1) * HD, jP:jP + P],
                            )
                        proj_chunk(t0 + j, tail=True,
                                   last_chunk=(j == TQ // P - 1))
                    return
                oc = stage.tile([P, 2, TQ], F32, tag="oc", bufs=2, name="oc")
                nc.vector.tensor_copy(oc[:], op_t[:])
                # custom DVE ops break on partition-shifted input: move the
                # denominator rows to base partition 0 with a standard copy
                rb0 = stage.tile([HD, 2, TQ], F32, tag="rb0", bufs=3, name="rb0")
                nc.vector.tensor_copy(rb0[:], oc[HD:2 * HD, :, :])
                rb = stage.tile([HD, 2, TQ], F32, tag="rb", bufs=3, name="rb")
                nc.vector.reciprocal_approx_fast(rb[:], rb0[:])
                for h in range(2):
                    nc.vector.tensor_mul(
                        ot_sb[h * HD:(h + 1) * HD, base:base + TQ],
                        oc[0:HD, h, :], rb[:, h, :],
                    )

            # ---- prologue: first K/Q chunks so the exp pipeline starts
            # early; k/q halves interleaved so each one runs as soon as
            # its own c-half of hr0 lands (wk/wq ship ahead of x)
            qk_half(WK, kt_sb, 0, 0)
            qk_half(WQ, qt_sb, 0, 0)
            qk_half(WK, kt_sb, 0, 1)
            qk_half(WQ, qt_sb, 0, 1)
            do_S(0)
            do_S(1)
            # broadcast the free-axis V bias across partitions (K=1 matmul)
            bb = psm.tile([P, CSL], F32, tag="mm", bufs=2, name="bb")
            nc.tensor.matmul(bb[:], ones1[0:1, 0:P], bv_row[:], start=True, stop=True)
            nc.vector.tensor_copy(bv_bc[:], bb[:])

            def qk2(s0, wi, dst, t8):
                return [(s0, lambda: qk_half(wi, dst, t8, 0)),
                        (s0 + 1, lambda: qk_half(wi, dst, t8, 1))]

            # one injected task per step at most in steady state; unit-
            # boundary steps (s%16 in {0,1}) always carry a task so the
            # PE has queued work while the drain's oc copy releases the
            # op banks. Deadlock rule: a chunk's last part must emit
            # before do_S of the first step that reads it (do_S(s+2) runs
            # before the pops of step s); v(i) at/before its first PV
            # step (pops precede the PV in-step).
            sched = []
            sched += [(i, lambda i=i: v_chunk(i)) for i in range(16)]
            sched += qk2(0, WK, kt_sb, 1)
            sched += qk2(4, WK, kt_sb, 2)
            sched += qk2(8, WK, kt_sb, 3)
            sched += qk2(10, WQ, qt_sb, 1)
            # qt2/qt3 ride the ramp steps 12-15: the PE idles there
            # waiting hr3 for kt3/S(12), and their own x (hr2/hr3) lands
            # in that window - executing them in the ramp shadow frees
            # steps 20-21/30-31 of the saturated steady-state for proj
            sched += qk2(12, WQ, qt_sb, 2)   # bias at 13 <= 27 deadline
            sched += qk2(14, WQ, qt_sb, 3)
            sched += qk2(16, WK, kt_sb, 4)   # boundary u1
            sched += qk2(18, WK, kt_sb, 5)
            sched += qk2(32, WQ, qt_sb, 4)   # boundary u2
            sched += [(34 + 2 * j, lambda i=16 + j: v_chunk(i)) for j in range(7)]
            sched += qk2(48, WQ, qt_sb, 5)   # boundary u3
            sched += [(50 + 2 * j, lambda i=23 + j: v_chunk(i)) for j in range(7)]
            sched += qk2(64, WK, kt_sb, 6)   # boundary u4 (bias 65 < do_S(72)@70)
            sched += [(66, lambda: v_chunk(30)), (67, lambda: v_chunk(31))]
            sched += qk2(68, WK, kt_sb, 7)   # bias 69 < do_S(76)@74
            sched += qk2(80, WQ, qt_sb, 6)   # boundary u5
            sched += qk2(96, WQ, qt_sb, 7)   # boundary u6
            sched += [(112, junk_fill), (113, junk_fill)]  # boundary u7
            sched.sort(key=lambda e: e[0])

            proj_q = []  # dynamic: projection sub-tasks appear after drains
            op_t = None
            si = 0
            for s in range(NS):
                u, k0 = divmod(s, NKC)
                b, qc = units[u]
                if k0 == 0:
                    op_t = psm.tile([P, 2, TQ], F32, tag="op", bufs=1, name="op_t")
                # S then exp lead the emission each step: ACT is the pacer
                # and an exp emitted behind injected proj/qkv PE work ends
                # up gated on that work completing
                if s + 2 < NS:
                    do_S(s + 2)
                pt = ptp.tile([P, 2, TQ], BF16, tag="pt", bufs=6, name="pt")
                with tc.high_priority():
                    nc.scalar.activation(pt[:], sp_tiles.pop(s)[:], Exp)
                popped = False
                while si < len(sched) and sched[si][0] <= s:
                    sched[si][1]()
                    si += 1
                    popped = True
                # proj strictly takes sched-free steps; the backlog drains
                # in the task-free zones
                if proj_q and not popped and 4 <= k0 < 14:
                    proj_q.pop(0)()
                for h in range(2):
                    nc.tensor.matmul(
                        op_t[:, h, :],
                        v_sb[:, b * TB + k0, h, :],
                        pt[:, h, :],
                        start=(k0 == 0), stop=(k0 == NKC - 1),
                    )
                if k0 == NKC - 1:
                    drain(op_t, u, last=(s == NS - 1))
                    if s != NS - 1:
                        t0 = (b * T + qc * TQ) // P
                        proj_q += [lambda t=t0 + j: proj_chunk(t) for j in range(TQ // P)]
            while proj_q:
                proj_q.pop(0)()

    nc.compile()
    return nc


def _shard_inputs(x, w_qkv, b_qkv, w_proj, b_proj):
    import ml_dtypes

    bf16 = ml_dtypes.bfloat16
    sc = np.float32(HD ** -0.5)
    # [hr, p, c, t'] with contiguous 1MB per-hr slices (8KB/partition rows)
    x2 = np.ascontiguousarray(
        x.reshape(NHR, TQ, NCC, P).transpose(0, 3, 2, 1).astype(bf16)
    ).reshape(NHR, P, NCC * TQ)

    def wprep(w):  # [C, CSL] -> SBUF layout [P, NCC, CSL], contiguous
        return np.ascontiguousarray(
            w.astype(bf16).reshape(NCC, P, CSL).transpose(1, 0, 2))

    in_maps = []
    for i in range(NCORE):
        h0 = 2 * i
        cs = slice(h0 * HD, h0 * HD + CSL)
        wqkv = np.stack([
            wprep(w_qkv[:, 0 * C:1 * C][:, cs] * sc),
            wprep(w_qkv[:, 1 * C:2 * C][:, cs]),
            wprep(w_qkv[:, 2 * C:3 * C][:, cs]),
        ], axis=1)  # [P, 3, NCC, CSL]
        bqk = np.stack([
            b_qkv[0 * C:1 * C][cs] * sc,
            b_qkv[1 * C:2 * C][cs],
        ], axis=1).astype(np.float32)  # [CSL, 2]
        m = {
            "x": x2,
            "wqkv": np.ascontiguousarray(wqkv),
            "wp": np.ascontiguousarray(w_proj[cs, :].astype(bf16)),
            "bqk": np.ascontiguousarray(bqk),
            "bv": np.ascontiguousarray(b_qkv[2 * C:3 * C][cs].reshape(1, CSL), dtype=np.float32),
        }
        in_maps.append(m)
    return in_maps


def _run(inputs, trace=False):
    from concourse.bass_utils import run_bass_kernel_spmd

    if "nc" not in _CACHE:
        _CACHE["nc"] = _build()
    nc = _CACHE["nc"]
    in_maps = _shard_inputs(
        np.asarray(inputs["x"]), np.asarray(inputs["w_qkv"]), np.asarray(inputs["b_qkv"]),
        np.asarray(inputs["w_proj"]), np.asarray(inputs["b_proj"]))
    res = run_bass_kernel_spmd(nc, in_maps, list(range(NCORE)), trace=trace)
    out = np.zeros((BT, C), dtype=np.float32)
    for i in range(NCORE):
        out += np.asarray(res.results[i]["out"]).astype(np.float32)
    out += np.asarray(inputs["b_proj"], dtype=np.float32)
    return out.reshape(B, T, C), res


def kernel(**inputs) -> np.ndarray:
    out, _ = _run(inputs, trace=False)
    return out
